# revision 55
# baseline (speedup 1.0000x reference)
"""Trainium2 Bass kernel for the 20-layer dilated-causal-conv audio model.

Formulation (validated against the reference in numpy):
- Only the last 128 output timesteps are needed -> per-layer suffix pyramid.
  Layer i only computes timesteps in blocks [TB[i+1], 512) of 16 steps each.
- Channels are tiny (8), so convs run on the TensorEngine as block-Toeplitz
  matmuls: partition dim = 16 timesteps x 8 channels = 128; each conv tap is a
  host-built 128x128 stationary matrix; taps accumulate in PSUM.
- Contraction-16 matmuls run at half rate on HW, so the HOST expands audio
  and ctrl to the 128-partition channel-broadcast layout (x0bc, C) and they
  DMA straight into SBUF; every device matmul is K=128. The per-layer
  control term is a cw_i*I matmul on C (the scaled identities ship in the
  weight pack), and layer 0's taps/residual read x0bc like any layer.
  Per-channel constants (conv bias, ctrl bias, folded io_b drift) ride the
  ReLU activation's per-partition bias.
- Residual 1x1 (io_w) is a block-diagonal matmul; the residual add runs on
  the VectorEngine. The final mixer is a per-layer [128,16] matmul over the
  last 8 blocks, accumulated in one PSUM group.
- Data parallel over batch: 32 batches -> 8 cores x 4; 2 chains per core;
  CHUNKB=96-block chunks wavefront-pipeline consecutive layers. Emission is
  phase-split per layer (all chains' conv matmuls, then acts, then io
  matmuls, then adds, mixer last) so ready PE work is never queued behind
  blocked PE work in the in-order engines.
- fp16 datapath: activations, inputs and the expanded weights are float16
  (PE: 1 cycle/row at any size vs f32r's 4x small-matmul penalty; halves
  weight DMA bytes). PSUM accumulation and biases stay fp32. Measured
  end-to-end relative error vs the fp32 reference: ~1.6e-3.
- All expanded 128-partition weights are host-packed into one [128, COLS]
  tensor streamed with a handful of large contiguous DMAs (per-DMA fixed
  costs ~1.2us dominate many-small-DMA schedules).
"""

import numpy as np

import concourse.bass as bass
import concourse.mybir as mybir
import concourse.tile as tile
from concourse.bass_utils import run_bass_kernel_spmd

# ---------------------------------------------------------------- constants
DIL = [1, 2, 4, 8, 16, 32, 64, 128, 256, 512] * 2
NL = 20          # layers
CH = 8           # channels
BLK = 16         # timesteps per block
NB = 512         # blocks in T=8192
T = 8192
B = 32           # total batch
NCORES = 8
BPC = B // NCORES  # batch per core
NCHAINS = 2      # independent batch chains (latency hiding)
CHUNKB = 96      # output blocks per chunk (PSUM bank limit: bpc_c*CHUNKB<=512)
NWDMA = 4        # weight-pack DMA chunks
CTRL_FIRST = True  # issue the ctrl/audio-broadcast matmuls before the taps
                   # (they don't depend on the layer chain -> off critical path)
PCBUFS = 3       # conv PSUM pool buffers (PSUM budget: PCBUFS+PIOBUFS+NCHAINS<=8)
PIOBUFS = 3      # residual PSUM pool buffers
HBUFS = 12       # h SBUF pool buffers
DMASP = 0        # issue all DMAs from the (otherwise idle) SP queue
XBUFS = 2        # x-stream SBUF pool buffers
MERGE_AT = 21    # merge the batch chains into one stream from this layer on
                 # (measured neutral-to-worse on HW; NL+1 = never merge)
SINGLETHR = 96   # layers with extent <= this get ONE chunk (no split overhead;
                 # PSUM bank limit: bpc_c*SINGLETHR*4B <= 2KB)

# dev-only ablation switches for HW cost attribution (subset of
# {"mm2", "act2", "dve2", "nowdma", "noctrl"}); empty for the real kernel
ABLATE = set()

DT = mybir.dt.float16     # datapath dtype (x, h, weights, audio, ctrl)
F32 = mybir.dt.float32

# block-start table: TB[i] = first block of x~_i ; TB[NL] = first output block.
# Extents (NB - TB[i]) are kept even (cheap, and matches the f32r-era layout).
TB = [0] * (NL + 1)
TB[NL] = NB - 8
for i in range(NL - 1, -1, -1):
    TB[i] = TB[i + 1] - max(1, (2 * DIL[i]) // BLK)
    if (NB - TB[i]) % 2:
        TB[i] -= 1

# per-layer tap block offsets
_TAP_OFFSETS = []
for _i in range(NL):
    d = DIL[_i]
    _TAP_OFFSETS.append([0, d // BLK, 2 * (d // BLK)] if d >= BLK else [0, 1])

# packed weight-tensor column layout: per layer [taps | cw_i*I | iow | mixw]
_COL_TAP = [0] * NL
_COL_CID = [0] * NL
_COL_IOW = [0] * NL
_COL_MIX = [0] * NL
_c = 0
for _i in range(NL):
    _COL_TAP[_i] = _c
    _c += len(_TAP_OFFSETS[_i]) * 128
    _COL_CID[_i] = _c
    _c += 128
    if _i < NL - 1:
        _COL_IOW[_i] = _c
        _c += 128
    _COL_MIX[_i] = _c
    _c += 16
COLS = _c

# weight DMA chunk boundaries: split at layer starts, roughly equal columns
_WSPLITS = [0]
for _i in range(1, NL):
    if _COL_TAP[_i] >= len(_WSPLITS) * COLS // NWDMA and len(_WSPLITS) < NWDMA:
        _WSPLITS.append(_COL_TAP[_i])
_WSPLITS.append(COLS)


# ------------------------------------------------- workaround: 1-wait limit
def _split_multi_waits(nc):
    """This walrus build allows only one sem wait per TPB instruction, but
    Tile's kernel-tail drain carries several. Move extras onto preceding
    same-engine nops (in-order execution keeps the gating semantics)."""
    tpb = {
        mybir.EngineType.SP,
        mybir.EngineType.PE,
        mybir.EngineType.DVE,
        mybir.EngineType.Activation,
        mybir.EngineType.Pool,
    }
    for f in nc.m.functions:
        for bb in f.blocks:
            new_list = []
            changed = False
            for inst in bb.instructions:
                si = inst.sync_info
                if si is not None and si.on_wait and len(si.on_wait) > 1 and inst.engine in tpb:
                    waits = list(si.on_wait)
                    for j, w in enumerate(waits[:-1]):
                        nop = mybir.InstNoOp(name=f"{inst.name}-ws{j}", ins=[], outs=[])
                        nop.engine = inst.engine
                        nop.sync_info = mybir.SyncInfo(on_wait=[w], on_update=[])
                        new_list.append(nop)
                    si.on_wait = waits[-1:]
                    changed = True
                new_list.append(inst)
            if changed:
                bb.instructions[:] = new_list


# ------------------------------------------------------------- host arrays
def _build_host_arrays(inputs):
    c_w0 = np.asarray(inputs["c_w0"], np.float32)    # [3,1,8]
    c_ws = np.asarray(inputs["c_ws"], np.float32)    # [19,3,8,8]
    c_b = np.asarray(inputs["c_b"], np.float32)      # [20,8]
    ctrl_w = np.asarray(inputs["ctrl_w"], np.float32)  # [20,1,1]
    ctrl_b = np.asarray(inputs["ctrl_b"], np.float32)  # [20,1]
    io_w = np.asarray(inputs["io_w"], np.float32)    # [19,8,8]
    io_b = np.asarray(inputs["io_b"], np.float32)    # [19,8]
    mix_w = np.asarray(inputs["mix_w"], np.float32)  # [160,1]

    wpk = np.zeros((128, COLS), np.float32)
    biases = np.zeros((128, NL), np.float32)

    const_i = np.zeros(CH, np.float32)
    for i in range(NL):
        w = c_w0 if i == 0 else c_ws[i - 1]          # [3, cin, 8]
        cin = w.shape[1]
        d = DIL[i]
        wD = [w[2], w[1], w[0]]                      # wD[l] multiplies x[t - l*d]
        bias = c_b[i] + ctrl_b[i][0]
        if cin == CH:
            bias = bias + np.einsum("kco,c->o", w, const_i)
        biases[:, i] = np.tile(bias, BLK)

        # layer 0 (cin=1) reads the broadcast x0bc tile (all 8 channel rows
        # carry the audio value; use channel row 0). All taps contract 128.
        def rows(ti):
            return slice(ti * 8, ti * 8 + 1) if cin == 1 else slice(ti * 8, ti * 8 + cin)

        c0 = _COL_TAP[i]
        if d >= BLK:
            for l in range(3):
                W = wpk[:, c0 + l * 128 : c0 + (l + 1) * 128]
                for t in range(BLK):
                    W[rows(t), t * 8 : t * 8 + 8] = wD[l][:cin]
        else:
            Wc = wpk[:, c0 : c0 + 128]
            Wp = wpk[:, c0 + 128 : c0 + 256]
            for to in range(BLK):
                for l in range(3):
                    ti = to - l * d
                    if ti >= 0:
                        Wc[rows(ti), to * 8 : to * 8 + 8] += wD[l][:cin]
                    else:
                        Wp[rows(ti + BLK), to * 8 : to * 8 + 8] += wD[l][:cin]

        for p in range(128):
            wpk[p, _COL_CID[i] + p] = ctrl_w[i][0, 0]
        for t in range(BLK):
            wpk[t * 8 : t * 8 + 8, _COL_MIX[i] + t] = mix_w[i * 8 : i * 8 + 8, 0]
        if i < NL - 1:
            for t in range(BLK):
                wpk[t * 8 : t * 8 + 8, _COL_IOW[i] + t * 8 : _COL_IOW[i] + t * 8 + 8] = io_w[i]
            const_i = const_i + io_b[i]

    return dict(
        wpk=wpk.astype(np.float16),
        biases=biases,
    )


# ----------------------------------------------------------- device program
_NC_CACHE = {}


def _build_nc(loop_k=None):
    """loop_k: dev-only probe mode — wrap the whole body in For_i(0, loop_k)
    so marginal per-iteration wall time on HW isolates kernel exec from the
    ~100ms dispatch floor."""
    nc = bass.Bass()
    bpc_c = BPC // NCHAINS          # batch elements per chain
    chunkb = min(CHUNKB, 512 // bpc_c)

    nblk0 = NB - TB[0]
    nblk1 = NB - TB[1]
    # audio/ctrl arrive host-blocked as [16=t-in-block, BPC, nblk]
    # audio/ctrl arrive host-expanded to the 128-partition broadcast layout
    # (partition = 16 timesteps x 8 channels, all channel rows carry the
    # signal) — the tiles are used directly as the layer-0 x~ stream / ctrl
    # broadcast C, with no on-device K=16 broadcast matmuls or PSUM copies.
    audio_h = nc.dram_tensor("audio", [128, BPC, nblk0], DT, kind="ExternalInput")
    ctrl_h = nc.dram_tensor("ctrl", [128, BPC, nblk1], DT, kind="ExternalInput")
    wpk_h = nc.dram_tensor("wpk", [128, COLS], DT, kind="ExternalInput")
    biases_h = nc.dram_tensor("biases", [128, NL], F32, kind="ExternalInput")
    out_h = nc.dram_tensor("out", [BPC, 128], F32, kind="ExternalOutput")

    import contextlib

    inline_k = 1
    if isinstance(loop_k, tuple):  # (outer For_i count, inline copies per pass)
        loop_k, inline_k = loop_k
    elif loop_k and loop_k < 0:    # negative: inline replication (no back-edge)
        inline_k, loop_k = -loop_k, None

    with tile.TileContext(nc) as tc:
        with (
            tc.For_i(0, loop_k, 1) if loop_k else contextlib.nullcontext(),
            tc.tile_pool(name="w", bufs=2) as wpool,
            tc.tile_pool(name="xs", bufs=XBUFS) as xpool,
            tc.tile_pool(name="h", bufs=HBUFS) as hpool,
            tc.tile_pool(name="pc", bufs=PCBUFS, space="PSUM") as pcpool,
            tc.tile_pool(name="pio", bufs=PIOBUFS, space="PSUM") as piopool,
            tc.tile_pool(name="pm", bufs=1, space="PSUM") as pmpool,
        ):
            for rep in range(inline_k):
                # DMA issue queues: SP is otherwise idle; keeping DMAs off the
                # Activation queue saves its sequencer ~667ns per dma_start
                q0 = nc.sync
                q1 = nc.sync if DMASP else nc.scalar
                # inputs first: layer 0 needs audio/ctrl before late weights
                x0bc = xpool.tile([128, BPC, nblk0], DT, tag="x_0", name="x0bc")
                cbc = xpool.tile([128, BPC, nblk1], DT, tag="cbc", name="cbc")
                q1.dma_start(out=x0bc[:], in_=audio_h[:])
                q0.dma_start(out=cbc[:], in_=ctrl_h[:])

                bias_t = wpool.tile([128, NL], F32, name="bias_t")
                q0.dma_start(out=bias_t[:], in_=biases_h[:])

                wpk_t = wpool.tile([128, COLS], DT, name="wpk_t")
                queues = [q0, q1]
                if "nowdma" not in ABLATE:
                    for qi in range(len(_WSPLITS) - 1):
                        a, b = _WSPLITS[qi], _WSPLITS[qi + 1]
                        queues[qi % 2].dma_start(
                            out=wpk_t[:, a:b], in_=wpk_h[:, a:b]
                        )

                pms = [
                    pmpool.tile([16, bpc_c, 8], F32, name=f"pms{c}", tag=f"pms{c}")
                    for c in range(NCHAINS)
                ]

                # streams: independent batch slices marching through the
                # layers. Pre-merge: one per chain (latency hiding). From
                # layer MERGE_AT on (small extents, overhead-dominated) the
                # chains merge into one full-batch stream — half the
                # instructions per layer.
                # Each stream: (x_tile, g0, nb) — x covers batches
                # [g0, g0+nb); bc tiles are full-batch, sliced via g0.
                streams = [(x0bc, c * bpc_c, bpc_c) for c in range(NCHAINS)]
                wm2 = "mm2" in ABLATE
                wa2 = "act2" in ABLATE
                wd2 = "dve2" in ABLATE
                for i in range(NL):
                    out_b = TB[i + 1]
                    nblk_out = NB - out_b
                    ntaps = len(_TAP_OFFSETS[i])
                    if nblk_out <= max(chunkb, SINGLETHR):
                        chunks = [(out_b, nblk_out)]
                    else:
                        chunks = []
                        hi = NB
                        while hi > out_b:
                            lo = max(out_b, hi - chunkb)
                            chunks.append((lo, hi - lo))
                            hi = lo
                        chunks = chunks[::-1]
                    ns = len(streams)
                    merge_next = (i + 1 >= MERGE_AT) and ns > 1
                    x_nexts = [None] * ns
                    if i < NL - 1:
                        if merge_next:
                            xm = xpool.tile(
                                [128, BPC, nblk_out], DT,
                                tag=f"xm_{i + 1}", name=f"xm_{i + 1}",
                            )
                            x_nexts = [xm] * ns
                        else:
                            for si, (_, g0, nb) in enumerate(streams):
                                x_nexts[si] = xpool.tile(
                                    [128, nb, nblk_out], DT,
                                    tag=f"x{si}_{i + 1}", name=f"x{si}_{i + 1}",
                                )
                    # phase-split emission: conv matmuls (ctrl cw_i*I opener +
                    # taps), acts, io matmuls, residual adds, mixer — ready PE
                    # work stays ahead of blocked work in the in-order queues.
                    pcs = [[None] * ns for _ in chunks]
                    hs = [[None] * ns for _ in chunks]
                    pios = [[None] * ns for _ in chunks]
                    for ci, (lo, w) in enumerate(chunks):
                        wm = 2 if wm2 else w
                        # all streams' ctrl openers first: they only need the
                        # input broadcast, so they fill the PE's wait on the
                        # residual adds instead of queuing behind blocked taps
                        for si, (x_t, g0, nb) in enumerate(streams):
                            pc = pcpool.tile([128, nb, max(chunkb, SINGLETHR)], F32, name="pc", tag="pc")
                            pcs[ci][si] = pc
                            a = lo - TB[1]
                            nc.tensor.matmul(
                                pc[:, :, :wm],
                                wpk_t[:, _COL_CID[i] : _COL_CID[i] + 128],
                                cbc[:, g0 : g0 + nb, a : a + wm],
                                start=True,
                                stop=False,
                            )
                        for si, (x_t, g0, nb) in enumerate(streams):
                            xs = slice(g0, g0 + nb) if i == 0 else slice(0, nb)
                            for j in range(ntaps):
                                off = _TAP_OFFSETS[i][j]
                                a = lo - off - TB[i]
                                nc.tensor.matmul(
                                    pcs[ci][si][:, :, :wm],
                                    wpk_t[:, _COL_TAP[i] + j * 128 : _COL_TAP[i] + (j + 1) * 128],
                                    x_t[:, xs, a : a + wm],
                                    start=False,
                                    stop=(j == ntaps - 1),
                                )
                    for ci, (lo, w) in enumerate(chunks):
                        wa = 2 if wa2 else w
                        for si, (x_t, g0, nb) in enumerate(streams):
                            h = hpool.tile([128, nb, max(chunkb, SINGLETHR)], DT, name="h")
                            hs[ci][si] = h
                            nc.scalar.activation(
                                out=h[:, :, :wa],
                                in_=pcs[ci][si][:, :, :wa],
                                func=mybir.ActivationFunctionType.Relu,
                                bias=bias_t[:, i : i + 1],
                                scale=1.0,
                            )
                    if i < NL - 1:
                        for ci, (lo, w) in enumerate(chunks):
                            wm = 2 if wm2 else w
                            for si, (x_t, g0, nb) in enumerate(streams):
                                pio = piopool.tile([128, nb, max(chunkb, SINGLETHR)], F32, name="pio")
                                pios[ci][si] = pio
                                nc.tensor.matmul(
                                    pio[:, :, :wm],
                                    wpk_t[:, _COL_IOW[i] : _COL_IOW[i] + 128],
                                    hs[ci][si][:, :, :wm],
                                    start=True,
                                    stop=True,
                                )
                        for ci, (lo, w) in enumerate(chunks):
                            wd = 2 if wd2 else w
                            for si, (x_t, g0, nb) in enumerate(streams):
                                xs = slice(g0, g0 + nb) if i == 0 else slice(0, nb)
                                ob = slice(g0, g0 + nb) if merge_next else slice(0, nb)
                                nc.vector.tensor_add(
                                    out=x_nexts[si][:, ob, lo - out_b : lo - out_b + wd],
                                    in0=x_t[:, xs, lo - TB[i] : lo - TB[i] + wd],
                                    in1=pios[ci][si][:, :, :wd],
                                )
                    lo, w = chunks[-1]  # rightmost chunk: mixer contribution
                    r = NB - 8 - lo
                    for c in range(NCHAINS):
                        si = c if len(streams) > 1 else 0
                        g0 = streams[si][1]
                        hb = c * bpc_c - g0
                        nc.tensor.matmul(
                            pms[c][:],
                            wpk_t[:, _COL_MIX[i] : _COL_MIX[i] + 16],
                            hs[-1][si][:, hb : hb + bpc_c, r : r + 8],
                            start=(i == 0),
                            stop=(i == NL - 1),
                            skip_group_check=True,
                        )
                    if i < NL - 1:
                        if merge_next:
                            streams = [(x_nexts[0], 0, BPC)]
                        else:
                            streams = [
                                (x_nexts[si], g0, nb)
                                for si, (_, g0, nb) in enumerate(streams)
                            ]

                # out: per chain [16, bpc_c, 8] -> DRAM [BPC, 128]
                for c in range(NCHAINS):
                    out_t = wpool.tile([16, bpc_c, 8], F32, name=f"out{c}", tag=f"out{c}")
                    nc.vector.tensor_copy(out=out_t[:], in_=pms[c][:])
                    dst = bass.AP(
                        tensor=out_h,
                        offset=c * bpc_c * 128,
                        ap=[[1, BLK], [128, bpc_c], [BLK, 8]],
                    )
                    nc.sync.dma_start(out=dst, in_=out_t[:])

    _split_multi_waits(nc)
    return nc


def _get_nc():
    if "nc" not in _NC_CACHE:
        _NC_CACHE["nc"] = _build_nc()
    return _NC_CACHE["nc"]


# ------------------------------------------------------------------- public
def _block(sig, b0):
    """[b, T] -> [128, b, nblk] suffix-block channel-broadcast layout starting
    at block b0 (partition = t*8+c within the block, all c rows = signal)."""
    nblk = NB - b0
    v = sig[:, b0 * BLK :].reshape(sig.shape[0], nblk, BLK)
    v = np.repeat(v.transpose(2, 0, 1), 8, axis=0)
    return np.ascontiguousarray(v).astype(np.float16)


def kernel(**inputs) -> np.ndarray:
    nc = _get_nc()
    host = _build_host_arrays(inputs)
    audio = np.asarray(inputs["audio"], np.float32)[:, :, 0]
    ctrl = np.asarray(inputs["ctrl"], np.float32)[:, :, 0]
    mix_b = float(np.asarray(inputs["mix_b"], np.float32)[0])

    in_maps = []
    for c in range(NCORES):
        sl = slice(c * BPC, (c + 1) * BPC)
        in_maps.append(
            {
                "audio": _block(audio[sl], TB[0]),
                "ctrl": _block(ctrl[sl], TB[1]),
                "wpk": host["wpk"],
                "biases": host["biases"],
            }
        )
    res = run_bass_kernel_spmd(nc, in_maps, core_ids=list(range(NCORES)))
    out = np.concatenate([res.results[c]["out"] for c in range(NCORES)], axis=0)
    return (out + mix_b).astype(np.float32)


# revision 61
# speedup vs baseline: 1.0799x; 1.0799x over previous
"""Trainium2 Bass kernel for the 20-layer dilated-causal-conv audio model.

Formulation (validated against the reference in numpy):
- Only the last 128 output timesteps are needed -> per-layer suffix pyramid.
  Layer i only computes timesteps in blocks [TB[i+1], 512) of 16 steps each.
- Channels are tiny (8), so convs run on the TensorEngine as block-Toeplitz
  matmuls: partition dim = 16 timesteps x 8 channels = 128; each conv tap is a
  host-built 128x128 stationary matrix; taps accumulate in PSUM.
- Contraction-16 matmuls run at half rate on HW, so the HOST expands audio
  and ctrl to the 128-partition channel-broadcast layout (x0bc, C) and they
  DMA straight into SBUF; every device matmul is K=128. The per-layer
  control term is a cw_i*I matmul on C (the scaled identities ship in the
  weight pack), and layer 0's taps/residual read x0bc like any layer.
  Per-channel constants (conv bias, ctrl bias, folded io_b drift) ride the
  ReLU activation's per-partition bias.
- Residual 1x1 (io_w) is a block-diagonal matmul; the residual add runs on
  the VectorEngine. The final mixer is a per-layer [128,16] matmul over the
  last 8 blocks, accumulated in one PSUM group.
- Data parallel over batch: 32 batches -> 8 cores x 4; 2 chains per core;
  CHUNKB=96-block chunks wavefront-pipeline consecutive layers. Emission is
  phase-split per layer (all chains' conv matmuls, then acts, then io
  matmuls, then adds, mixer last) so ready PE work is never queued behind
  blocked PE work in the in-order engines.
- fp16 datapath: activations, inputs and the expanded weights are float16
  (PE: 1 cycle/row at any size vs f32r's 4x small-matmul penalty; halves
  weight DMA bytes). PSUM accumulation and biases stay fp32. Measured
  end-to-end relative error vs the fp32 reference: ~1.6e-3.
- All expanded 128-partition weights are host-packed into one [128, COLS]
  tensor streamed with a handful of large contiguous DMAs (per-DMA fixed
  costs ~1.2us dominate many-small-DMA schedules).
"""

import numpy as np

import concourse.bass as bass
import concourse.mybir as mybir
import concourse.tile as tile
from concourse.bass_utils import run_bass_kernel_spmd

# ---------------------------------------------------------------- constants
DIL = [1, 2, 4, 8, 16, 32, 64, 128, 256, 512] * 2
NL = 20          # layers
CH = 8           # channels
BLK = 16         # timesteps per block
NB = 512         # blocks in T=8192
T = 8192
B = 32           # total batch
NCORES = 8
BPC = B // NCORES  # batch per core
NCHAINS = 2      # independent batch chains (latency hiding)
CHUNKB = 96      # output blocks per chunk (PSUM bank limit: bpc_c*CHUNKB<=512)
NWDMA = 4        # weight-pack DMA chunks
CTRL_FIRST = True  # issue the ctrl/audio-broadcast matmuls before the taps
                   # (they don't depend on the layer chain -> off critical path)
PCBUFS = 3       # conv PSUM pool buffers (PSUM budget: PCBUFS+PIOBUFS+NCHAINS<=8)
PIOBUFS = 3      # residual PSUM pool buffers
HBUFS = 12       # h SBUF pool buffers
DMASP = 0        # issue all DMAs from the (otherwise idle) SP queue
XBUFS = 2        # x-stream SBUF pool buffers
MERGE_AT = 21    # merge the batch chains into one stream from this layer on
                 # (measured neutral-to-worse on HW; NL+1 = never merge)
SINGLETHR = 96   # layers with extent <= this get ONE chunk (no split overhead;
                 # PSUM bank limit: bpc_c*SINGLETHR*4B <= 2KB)
CTRLX = 1        # emit all chains' ctrl openers before any chain's taps

# dev-only ablation switches for HW cost attribution (subset of
# {"mm2", "act2", "dve2", "nowdma", "noctrl"}); empty for the real kernel
ABLATE = set()

DT = mybir.dt.float16     # datapath dtype (x, h, weights, audio, ctrl)
F32 = mybir.dt.float32

# block-start table: TB[i] = first block of x~_i ; TB[NL] = first output block.
# Extents (NB - TB[i]) are kept even (cheap, and matches the f32r-era layout).
TB = [0] * (NL + 1)
TB[NL] = NB - 8
for i in range(NL - 1, -1, -1):
    TB[i] = TB[i + 1] - max(1, (2 * DIL[i]) // BLK)
    if (NB - TB[i]) % 2:
        TB[i] -= 1

# per-layer tap block offsets
_TAP_OFFSETS = []
for _i in range(NL):
    d = DIL[_i]
    _TAP_OFFSETS.append([0, d // BLK, 2 * (d // BLK)] if d >= BLK else [0, 1])

# packed weight-tensor column layout: per layer [taps | cw_i*I | iow | mixw]
_COL_TAP = [0] * NL
_COL_CID = [0] * NL
_COL_IOW = [0] * NL
_COL_MIX = [0] * NL
_c = 0
for _i in range(NL):
    _COL_TAP[_i] = _c
    _c += len(_TAP_OFFSETS[_i]) * 128
    _COL_CID[_i] = _c
    _c += 128
    if _i < NL - 1:
        _COL_IOW[_i] = _c
        _c += 128
    _COL_MIX[_i] = _c
    _c += 16
COLS = _c

# weight DMA chunk boundaries: split at layer starts, roughly equal columns
_WSPLITS = [0]
for _i in range(1, NL):
    if _COL_TAP[_i] >= len(_WSPLITS) * COLS // NWDMA and len(_WSPLITS) < NWDMA:
        _WSPLITS.append(_COL_TAP[_i])
_WSPLITS.append(COLS)


# ------------------------------------------------- workaround: 1-wait limit
def _split_multi_waits(nc):
    """This walrus build allows only one sem wait per TPB instruction, but
    Tile's kernel-tail drain carries several. Move extras onto preceding
    same-engine nops (in-order execution keeps the gating semantics)."""
    tpb = {
        mybir.EngineType.SP,
        mybir.EngineType.PE,
        mybir.EngineType.DVE,
        mybir.EngineType.Activation,
        mybir.EngineType.Pool,
    }
    for f in nc.m.functions:
        for bb in f.blocks:
            new_list = []
            changed = False
            for inst in bb.instructions:
                si = inst.sync_info
                if si is not None and si.on_wait and len(si.on_wait) > 1 and inst.engine in tpb:
                    waits = list(si.on_wait)
                    for j, w in enumerate(waits[:-1]):
                        nop = mybir.InstNoOp(name=f"{inst.name}-ws{j}", ins=[], outs=[])
                        nop.engine = inst.engine
                        nop.sync_info = mybir.SyncInfo(on_wait=[w], on_update=[])
                        new_list.append(nop)
                    si.on_wait = waits[-1:]
                    changed = True
                new_list.append(inst)
            if changed:
                bb.instructions[:] = new_list


# ------------------------------------------------------------- host arrays
def _build_host_arrays(inputs):
    c_w0 = np.asarray(inputs["c_w0"], np.float32)    # [3,1,8]
    c_ws = np.asarray(inputs["c_ws"], np.float32)    # [19,3,8,8]
    c_b = np.asarray(inputs["c_b"], np.float32)      # [20,8]
    ctrl_w = np.asarray(inputs["ctrl_w"], np.float32)  # [20,1,1]
    ctrl_b = np.asarray(inputs["ctrl_b"], np.float32)  # [20,1]
    io_w = np.asarray(inputs["io_w"], np.float32)    # [19,8,8]
    io_b = np.asarray(inputs["io_b"], np.float32)    # [19,8]
    mix_w = np.asarray(inputs["mix_w"], np.float32)  # [160,1]

    wpk = np.zeros((128, COLS), np.float32)
    biases = np.zeros((128, NL), np.float32)

    const_i = np.zeros(CH, np.float32)
    for i in range(NL):
        w = c_w0 if i == 0 else c_ws[i - 1]          # [3, cin, 8]
        cin = w.shape[1]
        d = DIL[i]
        wD = [w[2], w[1], w[0]]                      # wD[l] multiplies x[t - l*d]
        bias = c_b[i] + ctrl_b[i][0]
        if cin == CH:
            bias = bias + np.einsum("kco,c->o", w, const_i)
        biases[:, i] = np.tile(bias, BLK)

        # layer 0 (cin=1) reads the broadcast x0bc tile (all 8 channel rows
        # carry the audio value; use channel row 0). All taps contract 128.
        def rows(ti):
            return slice(ti * 8, ti * 8 + 1) if cin == 1 else slice(ti * 8, ti * 8 + cin)

        c0 = _COL_TAP[i]
        if d >= BLK:
            for l in range(3):
                W = wpk[:, c0 + l * 128 : c0 + (l + 1) * 128]
                for t in range(BLK):
                    W[rows(t), t * 8 : t * 8 + 8] = wD[l][:cin]
        else:
            Wc = wpk[:, c0 : c0 + 128]
            Wp = wpk[:, c0 + 128 : c0 + 256]
            for to in range(BLK):
                for l in range(3):
                    ti = to - l * d
                    if ti >= 0:
                        Wc[rows(ti), to * 8 : to * 8 + 8] += wD[l][:cin]
                    else:
                        Wp[rows(ti + BLK), to * 8 : to * 8 + 8] += wD[l][:cin]

        for p in range(128):
            wpk[p, _COL_CID[i] + p] = ctrl_w[i][0, 0]
        for t in range(BLK):
            wpk[t * 8 : t * 8 + 8, _COL_MIX[i] + t] = mix_w[i * 8 : i * 8 + 8, 0]
        if i < NL - 1:
            for t in range(BLK):
                wpk[t * 8 : t * 8 + 8, _COL_IOW[i] + t * 8 : _COL_IOW[i] + t * 8 + 8] = io_w[i]
            const_i = const_i + io_b[i]

    return dict(
        wpk=wpk.astype(np.float16),
        biases=biases,
    )


# ----------------------------------------------------------- device program
_NC_CACHE = {}


def _build_nc(loop_k=None):
    """loop_k: dev-only probe mode — wrap the whole body in For_i(0, loop_k)
    so marginal per-iteration wall time on HW isolates kernel exec from the
    ~100ms dispatch floor."""
    nc = bass.Bass()
    bpc_c = BPC // NCHAINS          # batch elements per chain
    chunkb = min(CHUNKB, 512 // bpc_c)

    nblk0 = NB - TB[0]
    nblk1 = NB - TB[1]
    # audio/ctrl arrive host-blocked as [16=t-in-block, BPC, nblk]
    # audio/ctrl arrive host-expanded to the 128-partition broadcast layout
    # (partition = 16 timesteps x 8 channels, all channel rows carry the
    # signal) — the tiles are used directly as the layer-0 x~ stream / ctrl
    # broadcast C, with no on-device K=16 broadcast matmuls or PSUM copies.
    audio_h = nc.dram_tensor("audio", [128, BPC, nblk0], DT, kind="ExternalInput")
    ctrl_h = nc.dram_tensor("ctrl", [128, BPC, nblk1], DT, kind="ExternalInput")
    wpk_h = nc.dram_tensor("wpk", [128, COLS], DT, kind="ExternalInput")
    biases_h = nc.dram_tensor("biases", [128, NL], F32, kind="ExternalInput")
    out_h = nc.dram_tensor("out", [BPC, 128], F32, kind="ExternalOutput")

    import contextlib

    inline_k = 1
    if isinstance(loop_k, tuple):  # (outer For_i count, inline copies per pass)
        loop_k, inline_k = loop_k
    elif loop_k and loop_k < 0:    # negative: inline replication (no back-edge)
        inline_k, loop_k = -loop_k, None

    with tile.TileContext(nc) as tc:
        with (
            tc.For_i(0, loop_k, 1) if loop_k else contextlib.nullcontext(),
            tc.tile_pool(name="w", bufs=2) as wpool,
            tc.tile_pool(name="xs", bufs=XBUFS) as xpool,
            tc.tile_pool(name="h", bufs=HBUFS) as hpool,
            tc.tile_pool(name="pc", bufs=PCBUFS, space="PSUM") as pcpool,
            tc.tile_pool(name="pio", bufs=PIOBUFS, space="PSUM") as piopool,
            tc.tile_pool(name="pm", bufs=1, space="PSUM") as pmpool,
        ):
            for rep in range(inline_k):
                # DMA issue queues: SP is otherwise idle; keeping DMAs off the
                # Activation queue saves its sequencer ~667ns per dma_start
                q0 = nc.sync
                q1 = nc.sync if DMASP else nc.scalar
                # inputs first: layer 0 needs audio/ctrl before late weights
                x0bc = xpool.tile([128, BPC, nblk0], DT, tag="x_0", name="x0bc")
                cbc = xpool.tile([128, BPC, nblk1], DT, tag="cbc", name="cbc")
                q1.dma_start(out=x0bc[:], in_=audio_h[:])
                q0.dma_start(out=cbc[:], in_=ctrl_h[:])

                bias_t = wpool.tile([128, NL], F32, name="bias_t")
                q0.dma_start(out=bias_t[:], in_=biases_h[:])

                wpk_t = wpool.tile([128, COLS], DT, name="wpk_t")
                queues = [q0, q1]
                if "nowdma" not in ABLATE:
                    for qi in range(len(_WSPLITS) - 1):
                        a, b = _WSPLITS[qi], _WSPLITS[qi + 1]
                        queues[qi % 2].dma_start(
                            out=wpk_t[:, a:b], in_=wpk_h[:, a:b]
                        )

                pms = [
                    pmpool.tile([16, bpc_c, 8], F32, name=f"pms{c}", tag=f"pms{c}")
                    for c in range(NCHAINS)
                ]

                # streams: independent batch slices marching through the
                # layers. Pre-merge: one per chain (latency hiding). From
                # layer MERGE_AT on (small extents, overhead-dominated) the
                # chains merge into one full-batch stream — half the
                # instructions per layer.
                # Each stream: (x_tile, g0, nb) — x covers batches
                # [g0, g0+nb); bc tiles are full-batch, sliced via g0.
                streams = [(x0bc, c * bpc_c, bpc_c) for c in range(NCHAINS)]
                wm2 = "mm2" in ABLATE
                wa2 = "act2" in ABLATE
                wd2 = "dve2" in ABLATE
                for i in range(NL):
                    out_b = TB[i + 1]
                    nblk_out = NB - out_b
                    ntaps = len(_TAP_OFFSETS[i])
                    if nblk_out <= max(chunkb, SINGLETHR):
                        chunks = [(out_b, nblk_out)]
                    else:
                        chunks = []
                        hi = NB
                        while hi > out_b:
                            lo = max(out_b, hi - chunkb)
                            chunks.append((lo, hi - lo))
                            hi = lo
                        chunks = chunks[::-1]
                    ns = len(streams)
                    merge_next = (i + 1 >= MERGE_AT) and ns > 1
                    x_nexts = [None] * ns
                    if i < NL - 1:
                        if merge_next:
                            xm = xpool.tile(
                                [128, BPC, nblk_out], DT,
                                tag=f"xm_{i + 1}", name=f"xm_{i + 1}",
                            )
                            x_nexts = [xm] * ns
                        else:
                            for si, (_, g0, nb) in enumerate(streams):
                                x_nexts[si] = xpool.tile(
                                    [128, nb, nblk_out], DT,
                                    tag=f"x{si}_{i + 1}", name=f"x{si}_{i + 1}",
                                )
                    # phase-split emission: conv matmuls (ctrl cw_i*I opener +
                    # taps), acts, io matmuls, residual adds, mixer — ready PE
                    # work stays ahead of blocked work in the in-order queues.
                    pcs = [[None] * ns for _ in chunks]
                    hs = [[None] * ns for _ in chunks]
                    pios = [[None] * ns for _ in chunks]
                    for ci, (lo, w) in enumerate(chunks):
                        wm = 2 if wm2 else w
                        # CTRLX: all streams' ctrl openers first — they only
                        # need the input broadcast, so they fill the PE's wait
                        # on the residual adds instead of queuing behind
                        # blocked taps
                        for si, (x_t, g0, nb) in enumerate(streams):
                            pc = pcpool.tile([128, nb, max(chunkb, SINGLETHR)], F32, name="pc", tag="pc")
                            pcs[ci][si] = pc
                            a = lo - TB[1]
                            nc.tensor.matmul(
                                pc[:, :, :wm],
                                wpk_t[:, _COL_CID[i] : _COL_CID[i] + 128],
                                cbc[:, g0 : g0 + nb, a : a + wm],
                                start=True,
                                stop=False,
                            )
                            if not CTRLX:
                                xs = slice(g0, g0 + nb) if i == 0 else slice(0, nb)
                                for j in range(ntaps):
                                    off = _TAP_OFFSETS[i][j]
                                    a = lo - off - TB[i]
                                    nc.tensor.matmul(
                                        pc[:, :, :wm],
                                        wpk_t[:, _COL_TAP[i] + j * 128 : _COL_TAP[i] + (j + 1) * 128],
                                        x_t[:, xs, a : a + wm],
                                        start=False,
                                        stop=(j == ntaps - 1),
                                    )
                        if CTRLX:
                            for si, (x_t, g0, nb) in enumerate(streams):
                                xs = slice(g0, g0 + nb) if i == 0 else slice(0, nb)
                                for j in range(ntaps):
                                    off = _TAP_OFFSETS[i][j]
                                    a = lo - off - TB[i]
                                    nc.tensor.matmul(
                                        pcs[ci][si][:, :, :wm],
                                        wpk_t[:, _COL_TAP[i] + j * 128 : _COL_TAP[i] + (j + 1) * 128],
                                        x_t[:, xs, a : a + wm],
                                        start=False,
                                        stop=(j == ntaps - 1),
                                    )
                    for ci, (lo, w) in enumerate(chunks):
                        wa = 2 if wa2 else w
                        for si, (x_t, g0, nb) in enumerate(streams):
                            h = hpool.tile([128, nb, max(chunkb, SINGLETHR)], DT, name="h")
                            hs[ci][si] = h
                            nc.scalar.activation(
                                out=h[:, :, :wa],
                                in_=pcs[ci][si][:, :, :wa],
                                func=mybir.ActivationFunctionType.Relu,
                                bias=bias_t[:, i : i + 1],
                                scale=1.0,
                            )
                    if i < NL - 1:
                        for ci, (lo, w) in enumerate(chunks):
                            wm = 2 if wm2 else w
                            for si, (x_t, g0, nb) in enumerate(streams):
                                pio = piopool.tile([128, nb, max(chunkb, SINGLETHR)], F32, name="pio")
                                pios[ci][si] = pio
                                nc.tensor.matmul(
                                    pio[:, :, :wm],
                                    wpk_t[:, _COL_IOW[i] : _COL_IOW[i] + 128],
                                    hs[ci][si][:, :, :wm],
                                    start=True,
                                    stop=True,
                                )
                        for ci, (lo, w) in enumerate(chunks):
                            wd = 2 if wd2 else w
                            for si, (x_t, g0, nb) in enumerate(streams):
                                xs = slice(g0, g0 + nb) if i == 0 else slice(0, nb)
                                ob = slice(g0, g0 + nb) if merge_next else slice(0, nb)
                                nc.vector.tensor_add(
                                    out=x_nexts[si][:, ob, lo - out_b : lo - out_b + wd],
                                    in0=x_t[:, xs, lo - TB[i] : lo - TB[i] + wd],
                                    in1=pios[ci][si][:, :, :wd],
                                )
                    lo, w = chunks[-1]  # rightmost chunk: mixer contribution
                    r = NB - 8 - lo
                    for c in range(NCHAINS):
                        si = c if len(streams) > 1 else 0
                        g0 = streams[si][1]
                        hb = c * bpc_c - g0
                        nc.tensor.matmul(
                            pms[c][:],
                            wpk_t[:, _COL_MIX[i] : _COL_MIX[i] + 16],
                            hs[-1][si][:, hb : hb + bpc_c, r : r + 8],
                            start=(i == 0),
                            stop=(i == NL - 1),
                            skip_group_check=True,
                        )
                    if i < NL - 1:
                        if merge_next:
                            streams = [(x_nexts[0], 0, BPC)]
                        else:
                            streams = [
                                (x_nexts[si], g0, nb)
                                for si, (_, g0, nb) in enumerate(streams)
                            ]

                # out: per chain [16, bpc_c, 8] -> DRAM [BPC, 128]
                for c in range(NCHAINS):
                    out_t = wpool.tile([16, bpc_c, 8], F32, name=f"out{c}", tag=f"out{c}")
                    nc.vector.tensor_copy(out=out_t[:], in_=pms[c][:])
                    dst = bass.AP(
                        tensor=out_h,
                        offset=c * bpc_c * 128,
                        ap=[[1, BLK], [128, bpc_c], [BLK, 8]],
                    )
                    nc.sync.dma_start(out=dst, in_=out_t[:])

    _split_multi_waits(nc)
    return nc


def _get_nc():
    if "nc" not in _NC_CACHE:
        _NC_CACHE["nc"] = _build_nc()
    return _NC_CACHE["nc"]


# ------------------------------------------------------------------- public
def _block(sig, b0):
    """[b, T] -> [128, b, nblk] suffix-block channel-broadcast layout starting
    at block b0 (partition = t*8+c within the block, all c rows = signal)."""
    nblk = NB - b0
    v = sig[:, b0 * BLK :].reshape(sig.shape[0], nblk, BLK)
    v = np.repeat(v.transpose(2, 0, 1), 8, axis=0)
    return np.ascontiguousarray(v).astype(np.float16)


def kernel(**inputs) -> np.ndarray:
    nc = _get_nc()
    host = _build_host_arrays(inputs)
    audio = np.asarray(inputs["audio"], np.float32)[:, :, 0]
    ctrl = np.asarray(inputs["ctrl"], np.float32)[:, :, 0]
    mix_b = float(np.asarray(inputs["mix_b"], np.float32)[0])

    in_maps = []
    for c in range(NCORES):
        sl = slice(c * BPC, (c + 1) * BPC)
        in_maps.append(
            {
                "audio": _block(audio[sl], TB[0]),
                "ctrl": _block(ctrl[sl], TB[1]),
                "wpk": host["wpk"],
                "biases": host["biases"],
            }
        )
    res = run_bass_kernel_spmd(nc, in_maps, core_ids=list(range(NCORES)))
    out = np.concatenate([res.results[c]["out"] for c in range(NCORES)], axis=0)
    return (out + mix_b).astype(np.float32)
